# revision 31
# baseline (speedup 1.0000x reference)
"""Trainium2 Bass kernel for a small dense transformer (Bigram model).

Model: B=4, T=2048, E=256, H=4 heads (HS=64), L=3 layers, V=32000 vocab.
logits = lm_head(trunk(tok_emb[idx] + pos_emb))  -> [4, 2048, 32000] f32.

Sharding over 8 NeuronCores: core c handles batch b = c//2 and vocab half
vh = c%2.  Each core runs the full trunk for its batch (replicated across
the pair -- trunk is small next to the lm_head) and then computes
logits[b, :, vh*16000:(vh+1)*16000] = x @ Wlm[:, half].

All matmul operands, weights, and the DRAM logits are bfloat16 (the host
upconverts the output to f32; rel tolerance is 2e-2, bf16 gives ~3e-3).
The f32 residual stream x stays in SBUF.

Layout strategy inside a core:
  - Residual stream x kept natural [t(128-part) x E] f32 in SBUF for LN.
  - LN output transposed via PE into hT [E(part) x T] bf16.
  - PSUM tiles are [128, 2, 512] f32 (2 banks); each matmul writes one
    512-col bank, drains/exps cover both banks in one instruction to
    amortize the per-instruction engine overheads.
  - Attention scores computed transposed S_T[s, t] = k . q over t-blocks
    of 1024 so softmax sum comes from a ones-column in v and P_T feeds
    the y^T matmul directly.  exp() has no max-subtraction (scores are
    O(1) by construction); diagonal blocks are masked by multiplying
    with a precomputed causal mask (DVE/Pool, bf16).
  - y accumulates per 512-col half in its own 1-bank PSUM tile so the
    two halves drain independently (recip + partition_broadcast + mul).
  - lm_head weights (bf16, 8.2 MB) are prefetched into SBUF during the
    trunk so the lm phase's DMA is output-writes only.
  - ln gains and the attention 1/sqrt(E) scale are folded into the weight
    matrices on the host.  All additive biases in this problem are zero;
    the lm bias is applied on the host if nonzero.
Engine split: PE matmuls; Act exp/sqrt/half-relu; DVE drains, residual
adds, y-normalize, half-relu; Pool xn, 1/l broadcast, causal masks.
"""

import numpy as np

P = 128
T = 2048
E = 256
H = 4
HS = 64
L = 3
V = 32000
VSH = V // 2  # vocab half per core
NT = T // P  # 16 token tiles of 128
TB = 1024  # attention t-block
NTB = T // TB  # 2
NLM = 500  # lm_head vocab tile
NLMT = VSH // NLM  # 32
EPS = 1e-5

_CACHE = {}


def _build_program(cfg=None):
    import concourse.bass as bass
    import concourse.mybir as mybir
    import concourse.tile as tile
    from concourse import bacc
    from concourse.masks import make_identity
    from contextlib import ExitStack

    cfg = cfg or {}
    a_psum_bufs = cfg.get("a_psum_bufs", 3)
    y_bufs = cfg.get("y_bufs", 2)
    pt_bufs = cfg.get("pt_bufs", 5)
    lm_bufs = cfg.get("lm_bufs", 3)
    ob_bufs = cfg.get("ob_bufs", 6)
    n_layers = cfg.get("n_layers", L)
    skip_lm = cfg.get("skip_lm", False)
    w_bufs = cfg.get("w_bufs", 1)
    obg = cfg.get("obg", 1)
    a_bufs = cfg.get("a_bufs", 1)
    work_bufs = cfg.get("work_bufs", 4)
    ypipe = cfg.get("ypipe", 3)

    F32 = mybir.dt.float32
    BF16 = mybir.dt.bfloat16
    I32 = mybir.dt.int32
    AF = mybir.ActivationFunctionType
    ALU = mybir.AluOpType

    nc = bacc.Bacc()
    idx32 = nc.declare_dram_parameter("idx32", [T], I32, isOutput=False)
    tok = nc.declare_dram_parameter("tok_emb", [V, E], BF16, isOutput=False)
    pos = nc.declare_dram_parameter("pos_emb", [T, E], BF16, isOutput=False)
    wq = nc.declare_dram_parameter("wq", [L, E, E], BF16, isOutput=False)
    wk = nc.declare_dram_parameter("wk", [L, E, E], BF16, isOutput=False)
    wv = nc.declare_dram_parameter("wv", [L, E, E], BF16, isOutput=False)
    wp = nc.declare_dram_parameter("wp", [L, E, E], BF16, isOutput=False)
    w1 = nc.declare_dram_parameter("w1", [L, E, 4 * E], BF16, isOutput=False)
    w2 = nc.declare_dram_parameter("w2", [L, 4 * E, E], BF16, isOutput=False)
    wlm = nc.declare_dram_parameter("wlm", [E, VSH], BF16, isOutput=False)
    out = nc.declare_dram_parameter("out", [T, VSH], BF16, isOutput=True)

    with tile.TileContext(nc) as tc, ExitStack() as ctx:
        const = ctx.enter_context(tc.tile_pool(name="const", bufs=1))
        res = ctx.enter_context(tc.tile_pool(name="res", bufs=1))
        res2 = ctx.enter_context(tc.tile_pool(name="res2", bufs=2))
        # lm weights: most chunks prefetched into SBUF during the trunk
        lm_resident = cfg.get("lm_resident", 10)
        lmw = ctx.enter_context(tc.tile_pool(name="lmw", bufs=lm_resident))
        # trunk-phase pools, closed before the lm head
        tk = ctx.enter_context(ExitStack())
        wpool = tk.enter_context(tc.tile_pool(name="wts", bufs=w_bufs))
        work = tk.enter_context(tc.tile_pool(name="work", bufs=work_bufs))
        apool = tk.enter_context(tc.tile_pool(name="apool", bufs=a_bufs))
        ppool = tk.enter_context(tc.tile_pool(name="ppool", bufs=pt_bufs))
        # PSUM budget: 8 banks = a 3x2 + y 2x1
        ps_a = tk.enter_context(tc.tile_pool(name="ps_a", bufs=a_psum_bufs, space="PSUM"))
        ps_y = tk.enter_context(tc.tile_pool(name="ps_y", bufs=y_bufs, space="PSUM"))

        identb = const.tile([P, P], BF16, tag="identb", name="identb")
        make_identity(nc, identb)
        identf = const.tile([P, P], F32, tag="identf", name="identf")
        make_identity(nc, identf)
        # causal mask for the diagonal 128x128 blocks: keep iff t >= s
        causal_m = const.tile([P, P], BF16, tag="cmask", name="cmask")
        nc.vector.memset(causal_m, 1.0)
        nc.gpsimd.affine_select(
            out=causal_m,
            in_=causal_m,
            compare_op=ALU.is_ge,
            fill=0.0,
            base=0,
            channel_multiplier=-1,
            pattern=[[1, P]],
        )
        eps_t = const.tile([P, 1], F32, tag="eps", name="eps")
        nc.vector.memset(eps_t, EPS)

        idx_sb = const.tile([P, NT], I32, tag="idx", name="idx_sb")
        nc.sync.dma_start(out=idx_sb, in_=idx32.rearrange("(n p) -> p n", p=P))

        def load_lm_chunk(nv2):
            wl = lmw.tile([P, 2, 2 * NLM], BF16, tag="wlm", name="wl")
            nc.sync.dma_start(
                out=wl,
                in_=wlm[:, nv2 * 2 * NLM : (nv2 + 1) * 2 * NLM].rearrange(
                    "(c p) n -> p c n", p=P
                ),
            )
            return wl

        # ---- embedding: x = tok_emb[idx] + pos_emb ----
        x_sb = res.tile([P, NT, E], F32, tag="x", name="x_sb")
        for g in range(2):
            n0 = g * 8
            xg = apool.tile([P, 8, E], BF16, tag="xg", name="xg")
            nc.gpsimd.indirect_dma_start(
                out=xg,
                out_offset=None,
                in_=tok[:],
                in_offset=bass.IndirectOffsetOnAxis(
                    ap=idx_sb[:, n0 : n0 + 8], axis=0
                ),
            )
            pt = apool.tile([P, 8, E], BF16, tag="pos", name="pos_t")
            nc.sync.dma_start(
                out=pt,
                in_=pos[n0 * P : (n0 + 8) * P].rearrange("(n p) e -> p n e", p=P),
            )
            for n4 in range(2):
                nc.vector.tensor_add(
                    out=x_sb[:, n0 + n4 * 4 : n0 + (n4 + 1) * 4, :],
                    in0=xg[:, n4 * 4 : (n4 + 1) * 4, :],
                    in1=pt[:, n4 * 4 : (n4 + 1) * 4, :],
                )

        def ln_transpose(dstT, do_ln=True):
            """LayerNorm x_sb (no affine: folded into weights) then write the
            transpose into dstT [P, 2, T] ([E-part, token] layout)."""
            for n in range(NT):
                if do_ln:
                    stats = work.tile([P, 6], F32, tag="stats", name="stats")
                    nc.vector.bn_stats(out=stats, in_=x_sb[:, n, :])
                    mv = work.tile([P, 2], F32, tag="mv", name="mv")
                    nc.vector.bn_aggr(out=mv, in_=stats)
                    std = work.tile([P, 1], F32, tag="std", name="std")
                    nc.scalar.activation(
                        out=std, in_=mv[:, 1:2], func=AF.Sqrt, bias=eps_t, scale=1.0
                    )
                    rstd = work.tile([P, 1], F32, tag="rstd", name="rstd")
                    nc.vector.reciprocal(out=rstd, in_=std)
                    xn = work.tile([P, E], BF16, tag="xn", name="xn")
                    nc.gpsimd.tensor_scalar(
                        out=xn,
                        in0=x_sb[:, n, :],
                        scalar1=mv[:, 0:1],
                        scalar2=rstd,
                        op0=ALU.subtract,
                        op1=ALU.mult,
                    )
                    src = xn
                else:
                    src = x_sb[:, n, :]
                pa = ps_a.tile([P, 2, 512], F32, tag="a", name="ps_tr")
                if do_ln:
                    pb = pa.bitcast(BF16)
                    ident = identb
                else:
                    pb = pa
                    ident = identf
                for c in range(2):
                    nc.tensor.transpose(
                        pb[:, c, :P], src[:, c * P : (c + 1) * P], ident
                    )
                if n % 2 == 0:
                    nc.vector.tensor_copy(
                        out=dstT[:, :, n * P : (n + 1) * P], in_=pb[:, :, :P]
                    )
                else:
                    nc.scalar.copy(
                        out=dstT[:, :, n * P : (n + 1) * P], in_=pb[:, :, :P]
                    )

        def ln_tile(n, dstT, drain_dve=True):
            """LN chain + transpose for one token tile (emits its own drains)."""
            stats = work.tile([P, 6], F32, tag="stats", name="stats")
            nc.vector.bn_stats(out=stats, in_=x_sb[:, n, :])
            mv = work.tile([P, 2], F32, tag="mv", name="mv")
            nc.vector.bn_aggr(out=mv, in_=stats)
            std = work.tile([P, 1], F32, tag="std", name="std")
            nc.scalar.activation(
                out=std, in_=mv[:, 1:2], func=AF.Sqrt, bias=eps_t, scale=1.0
            )
            rstd = work.tile([P, 1], F32, tag="rstd", name="rstd")
            nc.vector.reciprocal(out=rstd, in_=std)
            xn = work.tile([P, E], BF16, tag="xn", name="xn")
            nc.gpsimd.tensor_scalar(
                out=xn,
                in0=x_sb[:, n, :],
                scalar1=mv[:, 0:1],
                scalar2=rstd,
                op0=ALU.subtract,
                op1=ALU.mult,
            )
            pa = ps_a.tile([P, 2, 512], F32, tag="a", name="ps_tr")
            pb = pa.bitcast(BF16)
            for c in range(2):
                nc.tensor.transpose(pb[:, c, :P], xn[:, c * P : (c + 1) * P], identb)
            if drain_dve:
                nc.vector.tensor_copy(
                    out=dstT[:, :, n * P : (n + 1) * P], in_=pb[:, :, :P]
                )
            else:
                nc.scalar.copy(
                    out=dstT[:, :, n * P : (n + 1) * P], in_=pb[:, :, :P]
                )

        # ---- transformer layers, software-pipelined across t-blocks/layers ----
        # attention windows are Act(exp)-paced; all other PE work (qkv of the
        # next block/layer, proj/ln2/mlp of the previous block) is injected
        # into them as "fillers".
        lm_tiles = {}
        lm_queue = list(range(0 if skip_lm else NLMT // 2))

        def prefetch_lm(k):
            while lm_queue and len(lm_tiles) < min(lm_resident, k):
                nv2 = lm_queue.pop(0)
                lm_tiles[nv2] = load_lm_chunk(nv2)

        LW = {}  # (l, kind) -> weight tile

        def load_w(l, kinds):
            for kind in kinds:
                src, shape = {
                    "wq": (wq, [P, 2, E]),
                    "wk": (wk, [P, 2, E]),
                    "wv": (wv, [P, 2, E]),
                    "wp": (wp, [P, 2, E]),
                    "w1": (w1, [P, 2, 4 * E]),
                    "w2": (w2, [P, 8, E]),
                }[kind]
                t = wpool.tile(shape, BF16, tag=kind, name=f"{kind}_{l}")
                nc.sync.dma_start(
                    out=t, in_=src[l].rearrange("(c p) n -> p c n", p=P)
                )
                LW[(l, kind)] = t

        LT = {}  # per-layer lazily-created activation tiles

        def lt_get(l, key, mk):
            d = LT.setdefault(l, {})
            if key not in d:
                d[key] = mk()
            return d[key]

        def get_hT(l):
            return lt_get(l, "hT", lambda: res.tile([P, 2, T], BF16, tag="hT",
                                                    name=f"hT{l}"))

        def get_h2T(l):
            return lt_get(l, "h2T", lambda: res.tile([P, 2, T], BF16, tag="h2T",
                                                     name=f"h2T{l}"))

        def get_qT(l):
            return lt_get(l, "qT", lambda: res2.tile([P, 2, T], BF16, tag="qT",
                                                     name=f"qT{l}"))

        def get_kT(l):
            return lt_get(l, "kT", lambda: res2.tile([P, 2, T], BF16, tag="kT",
                                                     name=f"kT{l}"))

        def get_v(l):
            def mk():
                v_t = res2.tile([P, NT, H, HS + 1], BF16, tag="v", name=f"v{l}")
                nc.vector.memset(v_t[:, :, :, HS : HS + 1], 1.0)
                return v_t
            return lt_get(l, "v", mk)

        def get_yT(l):
            return lt_get(l, "yT", lambda: res.tile([P, 2, T], BF16, tag="yT",
                                                    name=f"yT{l}"))

        def get_aT(l, tb):
            return lt_get(l, ("aT", tb),
                          lambda: apool.tile([P, 8, TB], BF16, tag="aT", name="aT"))

        def qk_tile(l, w_kind, co, tb, dve):
            # q^T / k^T [E(part, 2 chunks), T]; head h: chunk h//2, rows (h%2)*64
            dstT = get_qT(l) if w_kind == "wq" else get_kT(l)
            w_sb = LW[(l, w_kind)]
            hT = get_hT(l)
            pa = ps_a.tile([P, 2, 512], F32, tag="a", name="ps_qk")
            for hh in range(2):
                for ci in range(2):
                    nc.tensor.matmul(
                        pa[:, hh, :],
                        lhsT=w_sb[:, ci, co * P : (co + 1) * P],
                        rhs=hT[:, ci, tb * TB + hh * 512 : tb * TB + (hh + 1) * 512],
                        start=(ci == 0),
                        stop=(ci == 1),
                    )
            eng = nc.vector.tensor_copy if dve else nc.scalar.copy
            eng(
                out=dstT[:, co, tb * TB : (tb + 1) * TB],
                in_=pa.rearrange("p a b -> p (a b)"),
            )

        def v_tile(l, n2, dve):
            v_sb = get_v(l)
            hT = get_hT(l)
            wv_sb = LW[(l, "wv")]
            pa = ps_a.tile([P, 2, 512], F32, tag="a", name="ps_v")
            for k in range(2):
                n = 2 * n2 + k
                for ci in range(2):
                    nc.tensor.matmul(
                        pa[:, k, :E],
                        lhsT=hT[:, ci, n * P : (n + 1) * P],
                        rhs=wv_sb[:, ci, :],
                        start=(ci == 0),
                        stop=(ci == 1),
                    )
            eng = nc.vector.tensor_copy if dve else nc.scalar.copy
            eng(
                out=v_sb[:, 2 * n2 : 2 * n2 + 2, :, 0:HS],
                in_=pa[:, :, :E].rearrange("p k (h d) -> p k h d", h=H),
            )

        def proj_pair(l, n2):
            yT = get_yT(l)
            wp_sb = LW[(l, "wp")]
            pa = ps_a.tile([P, 2, 512], F32, tag="a", name="ps_proj")
            for k in range(2):
                n = 2 * n2 + k
                for ci in range(2):
                    nc.tensor.matmul(
                        pa[:, k, :E],
                        lhsT=yT[:, ci, n * P : (n + 1) * P],
                        rhs=wp_sb[:, ci, :],
                        start=(ci == 0),
                        stop=(ci == 1),
                    )
            nc.vector.tensor_add(
                out=x_sb[:, 2 * n2 : 2 * n2 + 2, :],
                in0=x_sb[:, 2 * n2 : 2 * n2 + 2, :],
                in1=pa[:, :, :E],
            )

        def mlp1_tile(l, tb, m, dve):
            h2T = get_h2T(l)
            aT = get_aT(l, tb)
            w1_sb = LW[(l, "w1")]
            pa = ps_a.tile([P, 2, 512], F32, tag="a", name="ps_m1")
            for hh in range(2):
                for ci in range(2):
                    nc.tensor.matmul(
                        pa[:, hh, :],
                        lhsT=w1_sb[:, ci, m * P : (m + 1) * P],
                        rhs=h2T[:, ci, tb * TB + hh * 512 : tb * TB + (hh + 1) * 512],
                        start=(ci == 0),
                        stop=(ci == 1),
                    )
            if dve:
                nc.vector.tensor_scalar_max(
                    out=aT[:, m, :],
                    in0=pa.rearrange("p a b -> p (a b)"),
                    scalar1=0.0,
                )
            else:
                nc.scalar.activation(
                    out=aT[:, m, :],
                    in_=pa.rearrange("p a b -> p (a b)"),
                    func=AF.Relu,
                )

        def mlp2_tile(l, tb, k2):
            aT = get_aT(l, tb)
            w2_sb = LW[(l, "w2")]
            pa = ps_a.tile([P, 2, 512], F32, tag="a", name="ps_m2")
            for k in range(2):
                nk = k2 * 2 + k
                for m in range(8):
                    nc.tensor.matmul(
                        pa[:, k, :E],
                        lhsT=aT[:, m, nk * P : (nk + 1) * P],
                        rhs=w2_sb[:, m, :],
                        start=(m == 0),
                        stop=(m == 7),
                    )
            n0 = tb * 8 + k2 * 2
            nc.vector.tensor_add(
                out=x_sb[:, n0 : n0 + 2, :],
                in0=x_sb[:, n0 : n0 + 2, :],
                in1=pa[:, :, :E],
            )

        nmask = [0]

        def attn_block(l, tb, fillers):
            qT, kT, v_sb, yT = get_qT(l), get_kT(l), get_v(l), get_yT(l)
            t0 = tb * TB
            stride = None
            for h in range(H):
                kc, ko = h // 2, (h % 2) * HS
                nchunk = 8 * (tb + 1)
                if fillers and stride is None:
                    stride = max(1, (H * nchunk) // (len(fillers) + 1))
                py = [
                    ps_y.tile([HS + 1, 512], F32, tag="y", name="ps_yacc")
                    for _ in range(2)
                ]
                last = [8 * tb + 3, nchunk - 1]  # last live chunk per half
                pend = []  # [(si, pT, d0)] awaiting their y matmuls

                def drain_half(hh):
                    # normalize this 512-col half as soon as it stops
                    linv = work.tile([1, 512], BF16, tag="linv", name="linv")
                    with nc.allow_low_precision(reason="1/l softmax, tol 2e-2"):
                        nc.vector.reciprocal(out=linv, in_=py[hh][HS : HS + 1, :])
                    linb = work.tile([HS, 512], BF16, tag="linb", name="linb")
                    nc.gpsimd.partition_broadcast(linb, linv)
                    nc.vector.tensor_mul(
                        out=yT[ko : ko + HS, kc,
                               t0 + hh * 512 : t0 + (hh + 1) * 512],
                        in0=py[hh][0:HS, :],
                        in1=linb,
                    )

                def emit_y(si, pT, d0):
                    for hh in range(2):
                        c0 = max(d0 - hh * 512, 0)
                        if c0 >= 512:
                            continue
                        nc.tensor.matmul(
                            py[hh][:, c0:],
                            lhsT=v_sb[:, si, h, :],
                            rhs=pT[:, hh * 512 + c0 : (hh + 1) * 512],
                            start=(si == 0),
                            stop=(si == last[hh]),
                        )
                        if si == last[hh]:
                            drain_half(hh)

                for si in range(nchunk):
                    d0 = max(0, si * P - t0)  # first live col in block
                    pa = ps_a.tile([P, 2, 512], F32, tag="a", name="ps_s")
                    for hh in range(2):
                        c0 = max(d0 - hh * 512, 0)
                        if c0 >= 512:
                            continue
                        nc.tensor.matmul(
                            pa[:, hh, c0:],
                            lhsT=kT[ko : ko + HS, kc, si * P : (si + 1) * P],
                            rhs=qT[ko : ko + HS, kc,
                                   t0 + hh * 512 + c0 : t0 + (hh + 1) * 512],
                            start=True,
                            stop=True,
                        )
                    pT = ppool.tile([P, TB], BF16, tag="pT", name="pT")
                    nc.scalar.activation(
                        out=pT[:, d0:],
                        in_=pa.rearrange("p a b -> p (a b)")[:, d0:],
                        func=AF.Exp,
                    )
                    if si * P >= t0:
                        # mask the 128-col diagonal sub-block
                        meng = nc.vector if nmask[0] % 2 == 0 else nc.gpsimd
                        nmask[0] += 1
                        meng.tensor_mul(
                            out=pT[:, d0 : d0 + P],
                            in0=pT[:, d0 : d0 + P],
                            in1=causal_m,
                        )
                    if fillers and stride and (h * nchunk + si) % stride == stride - 1:
                        fillers.pop(0)()
                    pend.append((si, pT, d0))
                    if len(pend) > ypipe:
                        emit_y(*pend.pop(0))
                for e in pend:
                    emit_y(*e)
                pend.clear()
            # anything not injected runs after the block
            for f in fillers:
                f()
            fillers.clear()

        # prologue: layer 0 prep for t-block 0
        load_w(0, ["wq", "wk", "wv", "wp", "w1", "w2"])
        prefetch_lm(4)
        for n in range(8):
            ln_tile(n, get_hT(0), drain_dve=(n % 2 == 0))
        for w_kind in ("wq", "wk"):
            for co in range(2):
                qk_tile(0, w_kind, co, 0, dve=(co == 0))
        for n2 in range(4):
            v_tile(0, n2, dve=(n2 % 2 == 0))

        pending = []
        for l in range(n_layers):
            # attention tb0: fill with prev layer's tail + this layer's tb1 prep
            fill0 = pending
            fill0 += [lambda ll=l, n=n: ln_tile(n, get_hT(ll), drain_dve=True)
                      for n in range(8, NT)]
            fill0 += [lambda ll=l, w=w_kind, c=co: qk_tile(ll, w, c, 1, dve=True)
                      for w_kind in ("wq", "wk") for co in range(2)]
            fill0 += [lambda ll=l, n=n2: v_tile(ll, n, dve=True)
                      for n2 in range(4, 8)]
            attn_block(l, 0, fill0)
            if l + 1 < n_layers:
                load_w(l + 1, ["wq", "wk", "wv"])
                prefetch_lm(4 * (l + 2))

            # attention tb1: fill with tb0's proj/ln2/mlp + next layer's prep
            fill1 = [lambda ll=l, n=n2: proj_pair(ll, n) for n2 in range(4)]
            fill1 += [lambda ll=l, n=n: ln_tile(n, get_h2T(ll), drain_dve=True)
                      for n in range(8)]
            fill1 += [lambda ll=l, m=m: mlp1_tile(ll, 0, m, dve=True)
                      for m in range(8)]
            fill1 += [lambda ll=l, k=k2: mlp2_tile(ll, 0, k) for k2 in range(4)]
            if l + 1 < n_layers:
                fill1 += [lambda ll=l + 1, n=n: ln_tile(n, get_hT(ll),
                                                        drain_dve=True)
                          for n in range(8)]
                fill1 += [lambda ll=l + 1, w=w_kind, c=co:
                          qk_tile(ll, w, c, 0, dve=True)
                          for w_kind in ("wq", "wk") for co in range(2)]
                fill1 += [lambda ll=l + 1, n=n2: v_tile(ll, n, dve=True)
                          for n2 in range(4)]
            attn_block(l, 1, fill1)

            # inline: tb1 proj (frees yT before the next layer's attention)
            for n2 in range(4, 8):
                proj_pair(l, n2)

            # deferred tail, injected into the next attention window
            pending = [lambda ll=l, n=n: ln_tile(n, get_h2T(ll),
                                                 drain_dve=(n % 2 == 0))
                       for n in range(8, NT)]
            pending += [lambda ll=l, m=m: mlp1_tile(ll, 1, m, dve=(m % 2 == 1))
                        for m in range(8)]
            pending += [lambda ll=l, k=k2: mlp2_tile(ll, 1, k) for k2 in range(4)]
            if l + 1 < n_layers:
                pending.append(lambda ll=l + 1: load_w(ll, ["wp", "w1", "w2"]))
            else:
                for f in pending:
                    f()
                pending = []

        # ---- lm head ----
        xfT = res2.tile([P, 2, T], BF16, tag="qT", name="xfT")
        ln_transpose(xfT, do_ln=False)
        tk.close()
        opool = ctx.enter_context(tc.tile_pool(name="opool", bufs=ob_bufs))
        ps_lm = ctx.enter_context(tc.tile_pool(name="ps_lm", bufs=lm_bufs, space="PSUM"))
        for nv2 in range(0 if skip_lm else NLMT // 2):
            wl = lm_tiles[nv2]
            for g in range(NT // obg):
                if g == 1 and lm_queue:
                    # stream a non-resident chunk into the slot being freed
                    nxt = lm_queue.pop(0)
                    lm_tiles[nxt] = load_lm_chunk(nxt)
                ob = opool.tile([P, obg, 2 * NLM], BF16, tag="ob", name="ob")
                for k in range(obg):
                    n = g * obg + k
                    pa = ps_lm.tile([P, 2, 512], F32, tag="lm", name="ps_lm")
                    for j in range(2):
                        for ci in range(2):
                            nc.tensor.matmul(
                                pa[:, j, :NLM],
                                lhsT=xfT[:, ci, n * P : (n + 1) * P],
                                rhs=wl[:, ci, j * NLM : (j + 1) * NLM],
                                start=(ci == 0),
                                stop=(ci == 1),
                            )
                    dst = ob[:, k, :].rearrange("p (j n) -> p j n", j=2)
                    if (n + nv2) % 2 == 0:
                        nc.vector.tensor_copy(out=dst, in_=pa[:, :, :NLM])
                    else:
                        nc.scalar.copy(out=dst, in_=pa[:, :, :NLM])
                nc.sync.dma_start(
                    out=out[g * obg * P : (g + 1) * obg * P,
                            nv2 * 2 * NLM : (nv2 + 1) * 2 * NLM]
                    .rearrange("(k p) n -> p k n", p=P),
                    in_=ob,
                )

    nc.compile()
    return nc


TRACE = False
LAST_RESULT = None


def kernel(**inputs):
    import ml_dtypes
    from concourse.bass_utils import run_bass_kernel_spmd

    global LAST_RESULT
    BF = ml_dtypes.bfloat16

    idx = np.ascontiguousarray(np.asarray(inputs["idx"]).astype(np.int32))  # [4, T]
    tok_emb = np.asarray(inputs["tok_emb"], np.float32)
    pos_emb = np.asarray(inputs["pos_emb"], np.float32)
    Wq = np.asarray(inputs["Wq"], np.float32)
    Wk = np.asarray(inputs["Wk"], np.float32)
    Wv = np.asarray(inputs["Wv"], np.float32)
    Wproj = np.asarray(inputs["Wproj"], np.float32)
    bproj = np.asarray(inputs["bproj"], np.float32)
    ln1_g = np.asarray(inputs["ln1_g"], np.float32)
    ln1_b = np.asarray(inputs["ln1_b"], np.float32)
    W1 = np.asarray(inputs["W1"], np.float32)
    b1 = np.asarray(inputs["b1"], np.float32)
    W2 = np.asarray(inputs["W2"], np.float32)
    b2 = np.asarray(inputs["b2"], np.float32)
    ln2_g = np.asarray(inputs["ln2_g"], np.float32)
    ln2_b = np.asarray(inputs["ln2_b"], np.float32)
    Wlm = np.asarray(inputs["Wlm"], np.float32)
    blm = np.asarray(inputs["blm"], np.float32)

    # This kernel folds the LN affine into the weights; additive biases after
    # the matmuls are zero in this model (asserted).  The lm bias is applied
    # on the host if nonzero.
    for name, b in (("bproj", bproj), ("b1", b1), ("b2", b2)):
        assert np.all(b == 0.0), f"{name} must be zero for this kernel"
    for name, b in (("ln1_b", ln1_b), ("ln2_b", ln2_b)):
        assert np.all(b == 0.0), f"{name} must be zero for this kernel"

    scale = 1.0 / np.sqrt(np.float32(E))
    cvt = lambda a: np.ascontiguousarray(a.astype(BF))
    wq_f = cvt(ln1_g[:, :, None] * Wq * scale)  # [L,E,E]
    wk_f = cvt(ln1_g[:, :, None] * Wk)
    wv_f = cvt(ln1_g[:, :, None] * Wv)
    wp_f = cvt(Wproj)
    w1_f = cvt(ln2_g[:, :, None] * W1)
    w2_f = cvt(W2)

    if "nc" not in _CACHE:
        _CACHE["nc"] = _build_program()
    nc = _CACHE["nc"]

    common = {
        "tok_emb": cvt(tok_emb),
        "pos_emb": cvt(pos_emb),
        "wq": wq_f,
        "wk": wk_f,
        "wv": wv_f,
        "wp": wp_f,
        "w1": w1_f,
        "w2": w2_f,
    }
    wlm_bf = cvt(Wlm)
    in_maps = []
    for c in range(8):
        b, vh = c // 2, c % 2
        m = dict(common)
        m["idx32"] = np.ascontiguousarray(idx[b])
        m["wlm"] = np.ascontiguousarray(wlm_bf[:, vh * VSH : (vh + 1) * VSH])
        in_maps.append(m)

    r = run_bass_kernel_spmd(nc, in_maps, list(range(8)), trace=TRACE)
    LAST_RESULT = r

    B = idx.shape[0]
    logits = np.empty((B, T, V), np.float32)
    for c in range(8):
        b, vh = c // 2, c % 2
        logits[b, :, vh * VSH : (vh + 1) * VSH] = r.results[c]["out"].astype(
            np.float32
        )
    if np.any(blm != 0.0):
        logits += blm
    return logits


# revision 33
# speedup vs baseline: 1.0394x; 1.0394x over previous
"""Trainium2 Bass kernel for a small dense transformer (Bigram model).

Model: B=4, T=2048, E=256, H=4 heads (HS=64), L=3 layers, V=32000 vocab.
logits = lm_head(trunk(tok_emb[idx] + pos_emb))  -> [4, 2048, 32000] f32.

Sharding over 8 NeuronCores: core c handles batch b = c//2 and vocab half
vh = c%2.  Each core runs the full trunk for its batch (replicated across
the pair -- trunk is small next to the lm_head) and then computes
logits[b, :, vh*16000:(vh+1)*16000] = x @ Wlm[:, half].

All matmul operands, weights, and the DRAM logits are bfloat16 (the host
upconverts the output to f32; rel tolerance is 2e-2, bf16 gives ~3e-3).
The f32 residual stream x stays in SBUF.

Layout strategy inside a core:
  - Residual stream x kept natural [t(128-part) x E] f32 in SBUF for LN.
  - LN output transposed via PE into hT [E(part) x T] bf16.
  - PSUM tiles are [128, 2, 512] f32 (2 banks); each matmul writes one
    512-col bank, drains/exps cover both banks in one instruction to
    amortize the per-instruction engine overheads.
  - Attention scores computed transposed S_T[s, t] = k . q over t-blocks
    of 1024 so softmax sum comes from a ones-column in v and P_T feeds
    the y^T matmul directly.  exp() has no max-subtraction (scores are
    O(1) by construction); diagonal blocks are masked by multiplying
    with a precomputed causal mask (DVE/Pool, bf16).
  - y accumulates per 512-col half in its own 1-bank PSUM tile so the
    two halves drain independently (recip + partition_broadcast + mul).
  - lm_head weights (bf16, 8.2 MB) are prefetched into SBUF during the
    trunk so the lm phase's DMA is output-writes only.
  - ln gains and the attention 1/sqrt(E) scale are folded into the weight
    matrices on the host.  All additive biases in this problem are zero;
    the lm bias is applied on the host if nonzero.
Engine split: PE matmuls; Act exp/sqrt/half-relu; DVE drains, residual
adds, y-normalize, half-relu; Pool xn, 1/l broadcast, causal masks.
"""

import numpy as np

P = 128
T = 2048
E = 256
H = 4
HS = 64
L = 3
V = 32000
VSH = V // 2  # vocab half per core
NT = T // P  # 16 token tiles of 128
TB = 1024  # attention t-block
NTB = T // TB  # 2
NLM = 500  # lm_head vocab tile
NLMT = VSH // NLM  # 32
EPS = 1e-5

_CACHE = {}


def _build_program(cfg=None):
    import concourse.bass as bass
    import concourse.mybir as mybir
    import concourse.tile as tile
    from concourse import bacc
    from concourse.masks import make_identity
    from contextlib import ExitStack

    cfg = cfg or {}
    a_psum_bufs = cfg.get("a_psum_bufs", 3)
    y_bufs = cfg.get("y_bufs", 2)
    pt_bufs = cfg.get("pt_bufs", 5)
    lm_bufs = cfg.get("lm_bufs", 3)
    ob_bufs = cfg.get("ob_bufs", 6)
    n_layers = cfg.get("n_layers", L)
    skip_lm = cfg.get("skip_lm", False)
    w_bufs = cfg.get("w_bufs", 1)
    obg = cfg.get("obg", 1)
    a_bufs = cfg.get("a_bufs", 1)
    work_bufs = cfg.get("work_bufs", 4)
    ypipe = cfg.get("ypipe", 3)

    F32 = mybir.dt.float32
    BF16 = mybir.dt.bfloat16
    I32 = mybir.dt.int32
    AF = mybir.ActivationFunctionType
    ALU = mybir.AluOpType

    nc = bacc.Bacc()
    idx32 = nc.declare_dram_parameter("idx32", [T], I32, isOutput=False)
    tok = nc.declare_dram_parameter("tok_emb", [V, E], BF16, isOutput=False)
    pos = nc.declare_dram_parameter("pos_emb", [T, E], BF16, isOutput=False)
    wq = nc.declare_dram_parameter("wq", [L, E, E], BF16, isOutput=False)
    wk = nc.declare_dram_parameter("wk", [L, E, E], BF16, isOutput=False)
    wv = nc.declare_dram_parameter("wv", [L, E, E], BF16, isOutput=False)
    wp = nc.declare_dram_parameter("wp", [L, E, E], BF16, isOutput=False)
    w1 = nc.declare_dram_parameter("w1", [L, E, 4 * E], BF16, isOutput=False)
    w2 = nc.declare_dram_parameter("w2", [L, 4 * E, E], BF16, isOutput=False)
    wlm = nc.declare_dram_parameter("wlm", [E, VSH], BF16, isOutput=False)
    out = nc.declare_dram_parameter("out", [T, VSH], BF16, isOutput=True)

    with tile.TileContext(nc) as tc, ExitStack() as ctx:
        const = ctx.enter_context(tc.tile_pool(name="const", bufs=1))
        res = ctx.enter_context(tc.tile_pool(name="res", bufs=1))
        # lm weights: all 16 chunks live through the trunk (prefetch)
        lmw = ctx.enter_context(tc.tile_pool(name="lmw", bufs=NLMT // 2))
        # trunk-phase pools, closed before the lm head
        tk = ctx.enter_context(ExitStack())
        wpool = tk.enter_context(tc.tile_pool(name="wts", bufs=w_bufs))
        work = tk.enter_context(tc.tile_pool(name="work", bufs=work_bufs))
        apool = tk.enter_context(tc.tile_pool(name="apool", bufs=a_bufs))
        ppool = tk.enter_context(tc.tile_pool(name="ppool", bufs=pt_bufs))
        # PSUM budget: 8 banks = a 3x2 + y 2x1
        ps_a = tk.enter_context(tc.tile_pool(name="ps_a", bufs=a_psum_bufs, space="PSUM"))
        ps_y = tk.enter_context(tc.tile_pool(name="ps_y", bufs=y_bufs, space="PSUM"))

        identb = const.tile([P, P], BF16, tag="identb", name="identb")
        make_identity(nc, identb)
        identf = const.tile([P, P], F32, tag="identf", name="identf")
        make_identity(nc, identf)
        # causal mask for the diagonal 128x128 blocks: keep iff t >= s
        causal_m = const.tile([P, P], BF16, tag="cmask", name="cmask")
        nc.vector.memset(causal_m, 1.0)
        nc.gpsimd.affine_select(
            out=causal_m,
            in_=causal_m,
            compare_op=ALU.is_ge,
            fill=0.0,
            base=0,
            channel_multiplier=-1,
            pattern=[[1, P]],
        )
        eps_t = const.tile([P, 1], F32, tag="eps", name="eps")
        nc.vector.memset(eps_t, EPS)

        idx_sb = const.tile([P, NT], I32, tag="idx", name="idx_sb")
        nc.sync.dma_start(out=idx_sb, in_=idx32.rearrange("(n p) -> p n", p=P))

        def load_lm_chunk(nv2):
            wl = lmw.tile([P, 2, 2 * NLM], BF16, tag="wlm", name="wl")
            nc.sync.dma_start(
                out=wl,
                in_=wlm[:, nv2 * 2 * NLM : (nv2 + 1) * 2 * NLM].rearrange(
                    "(c p) n -> p c n", p=P
                ),
            )
            return wl

        # ---- embedding: x = tok_emb[idx] + pos_emb ----
        x_sb = res.tile([P, NT, E], F32, tag="x", name="x_sb")
        for g in range(2):
            n0 = g * 8
            pt = apool.tile([P, 8, E], BF16, tag="pos", name="pos_t")
            nc.sync.dma_start(
                out=pt,
                in_=pos[n0 * P : (n0 + 8) * P].rearrange("(n p) e -> p n e", p=P),
            )
            xg = apool.tile([P, 8, E], BF16, tag="xg", name="xg")
            for j in range(8):
                nc.gpsimd.indirect_dma_start(
                    out=xg[:, j, :],
                    out_offset=None,
                    in_=tok[:],
                    in_offset=bass.IndirectOffsetOnAxis(
                        ap=idx_sb[:, n0 + j : n0 + j + 1], axis=0
                    ),
                )
            for n4 in range(2):
                nc.vector.tensor_add(
                    out=x_sb[:, n0 + n4 * 4 : n0 + (n4 + 1) * 4, :],
                    in0=xg[:, n4 * 4 : (n4 + 1) * 4, :],
                    in1=pt[:, n4 * 4 : (n4 + 1) * 4, :],
                )

        def ln_transpose(dstT, do_ln=True):
            """LayerNorm x_sb (no affine: folded into weights) then write the
            transpose into dstT [P, 2, T] ([E-part, token] layout)."""
            for n in range(NT):
                if do_ln:
                    stats = work.tile([P, 6], F32, tag="stats", name="stats")
                    nc.vector.bn_stats(out=stats, in_=x_sb[:, n, :])
                    mv = work.tile([P, 2], F32, tag="mv", name="mv")
                    nc.vector.bn_aggr(out=mv, in_=stats)
                    std = work.tile([P, 1], F32, tag="std", name="std")
                    nc.scalar.activation(
                        out=std, in_=mv[:, 1:2], func=AF.Sqrt, bias=eps_t, scale=1.0
                    )
                    rstd = work.tile([P, 1], F32, tag="rstd", name="rstd")
                    nc.vector.reciprocal(out=rstd, in_=std)
                    xn = work.tile([P, E], BF16, tag="xn", name="xn")
                    nc.vector.tensor_scalar(
                        out=xn,
                        in0=x_sb[:, n, :],
                        scalar1=mv[:, 0:1],
                        scalar2=rstd,
                        op0=ALU.subtract,
                        op1=ALU.mult,
                    )
                    src = xn
                else:
                    src = x_sb[:, n, :]
                pa = ps_a.tile([P, 2, 512], F32, tag="a", name="ps_tr")
                if do_ln:
                    pb = pa.bitcast(BF16)
                    ident = identb
                else:
                    pb = pa
                    ident = identf
                for c in range(2):
                    nc.tensor.transpose(
                        pb[:, c, :P], src[:, c * P : (c + 1) * P], ident
                    )
                if n % 2 == 0:
                    nc.vector.tensor_copy(
                        out=dstT[:, :, n * P : (n + 1) * P], in_=pb[:, :, :P]
                    )
                else:
                    nc.scalar.copy(
                        out=dstT[:, :, n * P : (n + 1) * P], in_=pb[:, :, :P]
                    )

        def ln_tile(n, dstT, drain_dve=True):
            """LN chain + transpose for one token tile (emits its own drains)."""
            stats = work.tile([P, 6], F32, tag="stats", name="stats")
            nc.vector.bn_stats(out=stats, in_=x_sb[:, n, :])
            mv = work.tile([P, 2], F32, tag="mv", name="mv")
            nc.vector.bn_aggr(out=mv, in_=stats)
            std = work.tile([P, 1], F32, tag="std", name="std")
            nc.scalar.activation(
                out=std, in_=mv[:, 1:2], func=AF.Sqrt, bias=eps_t, scale=1.0
            )
            rstd = work.tile([P, 1], F32, tag="rstd", name="rstd")
            nc.vector.reciprocal(out=rstd, in_=std)
            xn = work.tile([P, E], BF16, tag="xn", name="xn")
            nc.vector.tensor_scalar(
                out=xn,
                in0=x_sb[:, n, :],
                scalar1=mv[:, 0:1],
                scalar2=rstd,
                op0=ALU.subtract,
                op1=ALU.mult,
            )
            pa = ps_a.tile([P, 2, 512], F32, tag="a", name="ps_tr")
            pb = pa.bitcast(BF16)
            for c in range(2):
                nc.tensor.transpose(pb[:, c, :P], xn[:, c * P : (c + 1) * P], identb)
            if drain_dve:
                nc.vector.tensor_copy(
                    out=dstT[:, :, n * P : (n + 1) * P], in_=pb[:, :, :P]
                )
            else:
                nc.scalar.copy(
                    out=dstT[:, :, n * P : (n + 1) * P], in_=pb[:, :, :P]
                )

        # ---- transformer layers ----
        lm_tiles = {}
        for l in range(n_layers):
            wq_sb = wpool.tile([P, 2, E], BF16, tag="wq", name="wq_sb")
            nc.sync.dma_start(out=wq_sb, in_=wq[l].rearrange("(c p) n -> p c n", p=P))
            wk_sb = wpool.tile([P, 2, E], BF16, tag="wk", name="wk_sb")
            nc.sync.dma_start(out=wk_sb, in_=wk[l].rearrange("(c p) n -> p c n", p=P))
            wv_sb = wpool.tile([P, 2, E], BF16, tag="wv", name="wv_sb")
            nc.sync.dma_start(out=wv_sb, in_=wv[l].rearrange("(c p) n -> p c n", p=P))
            wp_sb = wpool.tile([P, 2, E], BF16, tag="wp", name="wp_sb")
            nc.sync.dma_start(out=wp_sb, in_=wp[l].rearrange("(c p) n -> p c n", p=P))
            w1_sb = wpool.tile([P, 2, 4 * E], BF16, tag="w1", name="w1_sb")
            nc.sync.dma_start(out=w1_sb, in_=w1[l].rearrange("(c p) n -> p c n", p=P))
            w2_sb = wpool.tile([P, 8, E], BF16, tag="w2", name="w2_sb")
            nc.sync.dma_start(out=w2_sb, in_=w2[l].rearrange("(c p) n -> p c n", p=P))
            # prefetch lm chunks behind this layer's weights
            if not skip_lm:
                for nv2 in range(
                    (l * (NLMT // 2)) // n_layers, ((l + 1) * (NLMT // 2)) // n_layers
                ):
                    lm_tiles[nv2] = load_lm_chunk(nv2)

            hT = res.tile([P, 2, T], BF16, tag="hT", name="hT")
            for n in range(NT):
                ln_tile(n, hT, drain_dve=(n % 2 == 0))

            qT = res.tile([P, 2, T], BF16, tag="qT", name="qT")
            kT = res.tile([P, 2, T], BF16, tag="kT", name="kT")
            v_sb = res.tile([P, NT, H, HS + 1], BF16, tag="v", name="v_sb")
            nc.vector.memset(v_sb[:, :, :, HS : HS + 1], 1.0)

            def qk_tile(w_sb, dstT, co, tb, dve):
                # q^T / k^T [E(part, 2 chunks), T]; head h: chunk h//2, rows (h%2)*64
                pa = ps_a.tile([P, 2, 512], F32, tag="a", name="ps_qk")
                for hh in range(2):
                    for ci in range(2):
                        nc.tensor.matmul(
                            pa[:, hh, :],
                            lhsT=w_sb[:, ci, co * P : (co + 1) * P],
                            rhs=hT[:, ci, tb * TB + hh * 512 : tb * TB + (hh + 1) * 512],
                            start=(ci == 0),
                            stop=(ci == 1),
                        )
                eng = nc.vector.tensor_copy if dve else nc.scalar.copy
                eng(
                    out=dstT[:, co, tb * TB : (tb + 1) * TB],
                    in_=pa.rearrange("p a b -> p (a b)"),
                )

            def v_tile(n2, dve):
                pa = ps_a.tile([P, 2, 512], F32, tag="a", name="ps_v")
                for k in range(2):
                    n = 2 * n2 + k
                    for ci in range(2):
                        nc.tensor.matmul(
                            pa[:, k, :E],
                            lhsT=hT[:, ci, n * P : (n + 1) * P],
                            rhs=wv_sb[:, ci, :],
                            start=(ci == 0),
                            stop=(ci == 1),
                        )
                eng = nc.vector.tensor_copy if dve else nc.scalar.copy
                eng(
                    out=v_sb[:, 2 * n2 : 2 * n2 + 2, :, 0:HS],
                    in_=pa[:, :, :E].rearrange("p k (h d) -> p k h d", h=H),
                )

            yT = res.tile([P, 2, T], BF16, tag="yT", name="yT")

            def proj_pair(n2):
                pa = ps_a.tile([P, 2, 512], F32, tag="a", name="ps_proj")
                for k in range(2):
                    n = 2 * n2 + k
                    for ci in range(2):
                        nc.tensor.matmul(
                            pa[:, k, :E],
                            lhsT=yT[:, ci, n * P : (n + 1) * P],
                            rhs=wp_sb[:, ci, :],
                            start=(ci == 0),
                            stop=(ci == 1),
                        )
                nc.vector.tensor_add(
                    out=x_sb[:, 2 * n2 : 2 * n2 + 2, :],
                    in0=x_sb[:, 2 * n2 : 2 * n2 + 2, :],
                    in1=pa[:, :, :E],
                )

            h2T_box = []

            def get_h2T():
                if not h2T_box:
                    h2T_box.append(res.tile([P, 2, T], BF16, tag="h2T", name="h2T"))
                return h2T_box[0]

            aT_box = {}

            def mlp1_tile(tb, m, dve):
                h2T = get_h2T()
                if tb not in aT_box:
                    aT_box[tb] = apool.tile([P, 8, TB], BF16, tag="aT", name="aT")
                aT = aT_box[tb]
                pa = ps_a.tile([P, 2, 512], F32, tag="a", name="ps_m1")
                for hh in range(2):
                    for ci in range(2):
                        nc.tensor.matmul(
                            pa[:, hh, :],
                            lhsT=w1_sb[:, ci, m * P : (m + 1) * P],
                            rhs=h2T[:, ci, tb * TB + hh * 512 : tb * TB + (hh + 1) * 512],
                            start=(ci == 0),
                            stop=(ci == 1),
                        )
                if dve:
                    nc.vector.tensor_scalar_max(
                        out=aT[:, m, :],
                        in0=pa.rearrange("p a b -> p (a b)"),
                        scalar1=0.0,
                    )
                else:
                    nc.scalar.activation(
                        out=aT[:, m, :],
                        in_=pa.rearrange("p a b -> p (a b)"),
                        func=AF.Relu,
                    )

            def mlp2_tile(tb, k2):
                aT = aT_box[tb]
                pa = ps_a.tile([P, 2, 512], F32, tag="a", name="ps_m2")
                for k in range(2):
                    nk = k2 * 2 + k
                    for m in range(8):
                        nc.tensor.matmul(
                            pa[:, k, :E],
                            lhsT=aT[:, m, nk * P : (nk + 1) * P],
                            rhs=w2_sb[:, m, :],
                            start=(m == 0),
                            stop=(m == 7),
                        )
                n0 = tb * 8 + k2 * 2
                nc.vector.tensor_add(
                    out=x_sb[:, n0 : n0 + 2, :],
                    in0=x_sb[:, n0 : n0 + 2, :],
                    in1=pa[:, :, :E],
                )

            # attention, transposed-score flash style over 1024-col t-blocks.
            # The y matmul trails the score/exp of the NEXT chunk (software
            # pipeline) and `fillers` (other PE work) is injected into the
            # Act-paced stream.
            nmask = [0]

            def attn_block(tb, fillers):
                t0 = tb * TB
                stride = None
                for h in range(H):
                    kc, ko = h // 2, (h % 2) * HS
                    nchunk = 8 * (tb + 1)
                    if fillers and stride is None:
                        stride = max(1, (H * nchunk) // (len(fillers) + 1))
                    py = [
                        ps_y.tile([HS + 1, 512], F32, tag="y", name="ps_yacc")
                        for _ in range(2)
                    ]
                    last = [8 * tb + 3, nchunk - 1]  # last live chunk per half
                    pend = []  # [(si, pT, d0)] awaiting their y matmuls

                    def drain_half(hh):
                        # normalize this 512-col half as soon as it stops
                        linv = work.tile([1, 512], F32, tag="linv", name="linv")
                        with nc.allow_low_precision(reason="1/l softmax, tol 2e-2"):
                            nc.vector.reciprocal(out=linv, in_=py[hh][HS : HS + 1, :])
                        linb = work.tile([HS, 512], F32, tag="linb", name="linb")
                        nc.gpsimd.partition_broadcast(linb, linv)
                        nc.vector.tensor_mul(
                            out=yT[ko : ko + HS, kc,
                                   t0 + hh * 512 : t0 + (hh + 1) * 512],
                            in0=py[hh][0:HS, :],
                            in1=linb,
                        )

                    def emit_y(si, pT, d0):
                        for hh in range(2):
                            c0 = max(d0 - hh * 512, 0)
                            if c0 >= 512:
                                continue
                            nc.tensor.matmul(
                                py[hh][:, c0:],
                                lhsT=v_sb[:, si, h, :],
                                rhs=pT[:, hh * 512 + c0 : (hh + 1) * 512],
                                start=(si == 0),
                                stop=(si == last[hh]),
                            )
                            if si == last[hh]:
                                drain_half(hh)

                    for si in range(nchunk):
                        d0 = max(0, si * P - t0)  # first live col in block
                        pa = ps_a.tile([P, 2, 512], F32, tag="a", name="ps_s")
                        for hh in range(2):
                            c0 = max(d0 - hh * 512, 0)
                            if c0 >= 512:
                                continue
                            nc.tensor.matmul(
                                pa[:, hh, c0:],
                                lhsT=kT[ko : ko + HS, kc, si * P : (si + 1) * P],
                                rhs=qT[ko : ko + HS, kc,
                                       t0 + hh * 512 + c0 : t0 + (hh + 1) * 512],
                                start=True,
                                stop=True,
                            )
                        pT = ppool.tile([P, TB], BF16, tag="pT", name="pT")
                        nc.scalar.activation(
                            out=pT[:, d0:],
                            in_=pa.rearrange("p a b -> p (a b)")[:, d0:],
                            func=AF.Exp,
                        )
                        if si * P >= t0:
                            # mask the 128-col diagonal sub-block
                            meng = nc.vector if nmask[0] % 2 == 0 else nc.gpsimd
                            nmask[0] += 1
                            meng.tensor_mul(
                                out=pT[:, d0 : d0 + P],
                                in0=pT[:, d0 : d0 + P],
                                in1=causal_m,
                            )
                        if fillers and stride and (h * nchunk + si) % stride == stride - 1:
                            fillers.pop(0)()
                        pend.append((si, pT, d0))
                        if len(pend) > ypipe:
                            emit_y(*pend.pop(0))
                    for e in pend:
                        emit_y(*e)
                    pend.clear()
                # anything not injected runs after the block
                for f in fillers:
                    f()
                fillers.clear()

            # qkv for t-block 0 (+ all of k/v chunks 0..7)
            for w_sb, dstT in ((wq_sb, qT), (wk_sb, kT)):
                for co in range(2):
                    qk_tile(w_sb, dstT, co, 0, dve=(co == 0))
            for n2 in range(4):
                v_tile(n2, dve=(n2 % 2 == 0))

            # attention tb0, filling PE bubbles with tb1's qkv
            fill0 = []
            for w_sb, dstT in ((wq_sb, qT), (wk_sb, kT)):
                for co in range(2):
                    fill0.append(
                        lambda w=w_sb, d=dstT, c=co: qk_tile(w, d, c, 1, dve=True)
                    )
            for n2 in range(4, 8):
                fill0.append(lambda n=n2: v_tile(n, dve=True))
            attn_block(0, fill0)

            # attention tb1, filling with proj/ln2/mlp1 of t-block 0
            fill1 = [lambda n=n2: proj_pair(n) for n2 in range(4)]
            fill1 += [lambda n=n: ln_tile(n, get_h2T(), drain_dve=True)
                      for n in range(8)]
            fill1 += [lambda m=m: mlp1_tile(0, m, dve=True) for m in range(8)]
            attn_block(1, fill1)

            # tail: rest of t-block 0's mlp, then all of t-block 1
            for k2 in range(4):
                mlp2_tile(0, k2)
            for n2 in range(4, 8):
                proj_pair(n2)
            for n in range(8, NT):
                ln_tile(n, get_h2T(), drain_dve=(n % 2 == 0))
            for m in range(8):
                mlp1_tile(1, m, dve=(m % 2 == 1))
            for k2 in range(4):
                mlp2_tile(1, k2)

        # ---- lm head ----
        xfT = res.tile([P, 2, T], BF16, tag="xfT", name="xfT")
        ln_transpose(xfT, do_ln=False)
        tk.close()
        opool = ctx.enter_context(tc.tile_pool(name="opool", bufs=ob_bufs))
        ps_lm = ctx.enter_context(tc.tile_pool(name="ps_lm", bufs=lm_bufs, space="PSUM"))
        for nv2 in range(0 if skip_lm else NLMT // 2):
            wl = lm_tiles[nv2]
            for g in range(NT // obg):
                ob = opool.tile([P, obg, 2 * NLM], BF16, tag="ob", name="ob")
                for k in range(obg):
                    n = g * obg + k
                    pa = ps_lm.tile([P, 2, 512], F32, tag="lm", name="ps_lm")
                    for j in range(2):
                        for ci in range(2):
                            nc.tensor.matmul(
                                pa[:, j, :NLM],
                                lhsT=xfT[:, ci, n * P : (n + 1) * P],
                                rhs=wl[:, ci, j * NLM : (j + 1) * NLM],
                                start=(ci == 0),
                                stop=(ci == 1),
                            )
                    dst = ob[:, k, :].rearrange("p (j n) -> p j n", j=2)
                    if (n + nv2) % 2 == 0:
                        nc.vector.tensor_copy(out=dst, in_=pa[:, :, :NLM])
                    else:
                        nc.scalar.copy(out=dst, in_=pa[:, :, :NLM])
                nc.sync.dma_start(
                    out=out[g * obg * P : (g + 1) * obg * P,
                            nv2 * 2 * NLM : (nv2 + 1) * 2 * NLM]
                    .rearrange("(k p) n -> p k n", p=P),
                    in_=ob,
                )

    nc.compile()
    return nc


TRACE = False
LAST_RESULT = None


def kernel(**inputs):
    import ml_dtypes
    from concourse.bass_utils import run_bass_kernel_spmd

    global LAST_RESULT
    BF = ml_dtypes.bfloat16

    idx = np.ascontiguousarray(np.asarray(inputs["idx"]).astype(np.int32))  # [4, T]
    tok_emb = np.asarray(inputs["tok_emb"], np.float32)
    pos_emb = np.asarray(inputs["pos_emb"], np.float32)
    Wq = np.asarray(inputs["Wq"], np.float32)
    Wk = np.asarray(inputs["Wk"], np.float32)
    Wv = np.asarray(inputs["Wv"], np.float32)
    Wproj = np.asarray(inputs["Wproj"], np.float32)
    bproj = np.asarray(inputs["bproj"], np.float32)
    ln1_g = np.asarray(inputs["ln1_g"], np.float32)
    ln1_b = np.asarray(inputs["ln1_b"], np.float32)
    W1 = np.asarray(inputs["W1"], np.float32)
    b1 = np.asarray(inputs["b1"], np.float32)
    W2 = np.asarray(inputs["W2"], np.float32)
    b2 = np.asarray(inputs["b2"], np.float32)
    ln2_g = np.asarray(inputs["ln2_g"], np.float32)
    ln2_b = np.asarray(inputs["ln2_b"], np.float32)
    Wlm = np.asarray(inputs["Wlm"], np.float32)
    blm = np.asarray(inputs["blm"], np.float32)

    # This kernel folds the LN affine into the weights; additive biases after
    # the matmuls are zero in this model (asserted).  The lm bias is applied
    # on the host if nonzero.
    for name, b in (("bproj", bproj), ("b1", b1), ("b2", b2)):
        assert np.all(b == 0.0), f"{name} must be zero for this kernel"
    for name, b in (("ln1_b", ln1_b), ("ln2_b", ln2_b)):
        assert np.all(b == 0.0), f"{name} must be zero for this kernel"

    scale = 1.0 / np.sqrt(np.float32(E))
    cvt = lambda a: np.ascontiguousarray(a.astype(BF))
    wq_f = cvt(ln1_g[:, :, None] * Wq * scale)  # [L,E,E]
    wk_f = cvt(ln1_g[:, :, None] * Wk)
    wv_f = cvt(ln1_g[:, :, None] * Wv)
    wp_f = cvt(Wproj)
    w1_f = cvt(ln2_g[:, :, None] * W1)
    w2_f = cvt(W2)

    if "nc" not in _CACHE:
        _CACHE["nc"] = _build_program()
    nc = _CACHE["nc"]

    common = {
        "tok_emb": cvt(tok_emb),
        "pos_emb": cvt(pos_emb),
        "wq": wq_f,
        "wk": wk_f,
        "wv": wv_f,
        "wp": wp_f,
        "w1": w1_f,
        "w2": w2_f,
    }
    wlm_bf = cvt(Wlm)
    in_maps = []
    for c in range(8):
        b, vh = c // 2, c % 2
        m = dict(common)
        m["idx32"] = np.ascontiguousarray(idx[b])
        m["wlm"] = np.ascontiguousarray(wlm_bf[:, vh * VSH : (vh + 1) * VSH])
        in_maps.append(m)

    r = run_bass_kernel_spmd(nc, in_maps, list(range(8)), trace=TRACE)
    LAST_RESULT = r

    B = idx.shape[0]
    logits = np.empty((B, T, V), np.float32)
    for c in range(8):
        b, vh = c // 2, c % 2
        logits[b, :, vh * VSH : (vh + 1) * VSH] = r.results[c]["out"].astype(
            np.float32
        )
    if np.any(blm != 0.0):
        logits += blm
    return logits
